# revision 7
# baseline (speedup 1.0000x reference)
"""MaxK-GCN conv on 8 Trainium2 NeuronCores.

Pipeline (per core c, SPMD over 8 cores; nodes sharded 8 x 12500):
  host: load-balancing permutations — dst nodes LPT-packed into 98 blocks
        (~2045 in-edges each), src nodes greedy-packed into 8 slab-quarters
        so every (slab, block) bucket holds <=256 edges (2 gather tiles).
  phase 1: h = featT_c.T @ W (PE, bf16), top-16-of-64 threshold mask (DVE
           max8 + match_replace), scale by (max(out_deg,1)*max(in_deg,1))^-0.5,
           split each fp32 row into a [hi|lo] bf16 pair -> local table shard
           [12544, 128] bf16 (hi+lo reconstructs fp32 to ~2^-17).
  AllGather the table in 8 slab chunks so phase-2 gathers start after the
  first chunk (~45us) instead of after the full table.
  phase 2: per (pass, slab) group in slab-diagonal order (work tracks
           AllGather availability): dma_gather src rows (SWDGE), one-hot S
           tiles (scalar-engine broadcast-copy + DVE is_eq, keeping DVE in
           2-byte packed mode), matmul S^T @ G per 128-dst block in PSUM,
           evict per-group partial sums into an SBUF accumulator (decouples
           PSUM from pass completion), fold hi+lo + bias after the last
           slab, DMA out.

Edge bookkeeping (permutations, sort, padding, degree counts) is host-side
index metadata; all floating-point math runs on device.
"""
import sys
import os

sys.path.insert(0, "/opt/trn_rl_repo")

import numpy as np
import ml_dtypes
import concourse.bacc as bacc
import concourse.mybir as mybir
import concourse.tile as tile
from concourse.bass_utils import run_bass_kernel_spmd

P = 128
N_NODES = 100000
IN_FEATS = 256
OUT_FEATS = 64
N_CORES = 8
SHARD = N_NODES // N_CORES          # 12500 real nodes per core
SHARD_PAD = 12544                   # 98 * 128
N_BLOCKS = SHARD_PAD // P           # 98
N_SLABS = 8
QROWS = SHARD_PAD // N_SLABS        # 1568 rows per shard-slab
SLAB_ROWS = N_CORES * QROWS         # 12544 rows per slab-table
PASS_BLOCKS = [4] * 24 + [2]        # blocks per PSUM pass
N_PASSES = len(PASS_BLOCKS)
TABLE_ROWS = N_CORES * SHARD_PAD    # 100352
PAIR = 2 * OUT_FEATS                # 128 bf16 per table row (hi|lo)
CTMAX = 16                          # max tiles per (pass, slab) gather call
DIAG_G = 10                         # diagonal lag: calls per AllGather gap
SW = 4                              # one-hot build batch width
NEG_INF = -3.0e38

PASS_OF_BLOCK = np.repeat(np.arange(N_PASSES), PASS_BLOCKS)
PASS_BASE = np.cumsum([0] + PASS_BLOCKS[:-1])


def _balance_perms(src, dst, in_deg, out_deg):
    """Load-balancing layout: per-core dst->block LPT + src->slab greedy.

    Returns (dst_pos, src_pos): position of each node within its core's
    padded shard, for the output rows (dst_pos) and table rows (src_pos).
    """
    import heapq

    dst_pos = np.zeros(N_NODES, dtype=np.int64)
    for c in range(N_CORES):
        lo = c * SHARD
        deg = in_deg[lo:lo + SHARD]
        order = np.argsort(-deg, kind="stable")
        loads = np.zeros(N_BLOCKS, dtype=np.int64)
        slots = np.zeros(N_BLOCKS, dtype=np.int64)
        heap = [(0, b) for b in range(N_BLOCKS)]
        heapq.heapify(heap)
        pos = np.empty(SHARD, dtype=np.int64)
        for v in order:
            while True:
                _, b = heapq.heappop(heap)
                if slots[b] < P:
                    break
            pos[v] = b * P + slots[b]
            slots[b] += 1
            loads[b] += deg[v]
            if slots[b] < P:
                heapq.heappush(heap, (loads[b], b))
        dst_pos[lo:lo + SHARD] = pos

    # src->slab greedy: balance each consumer bucket (dst core, block)
    # across the 8 slab-tables; cells <= 256 keep buckets at 2 tiles.
    edge_bucket = ((dst // SHARD) * N_BLOCKS + (dst_pos[dst] // P)).astype(np.int64)
    nbkt = N_CORES * N_BLOCKS
    cell = np.zeros((nbkt, N_SLABS), dtype=np.int32)
    cap = 2 * P
    src_slab = np.zeros(N_NODES, dtype=np.int8)
    order_e = np.argsort(src, kind="stable")
    sb = edge_bucket[order_e]
    s_sorted = src[order_e]
    starts = np.searchsorted(s_sorted, np.arange(N_NODES))
    ends = np.searchsorted(s_sorted, np.arange(N_NODES) + 1)
    for c in range(N_CORES):
        lo = c * SHARD
        node_order = np.argsort(-out_deg[lo:lo + SHARD], kind="stable") + lo
        qcap = np.full(N_SLABS, QROWS, dtype=np.int64)
        for v in node_order:
            bkts = sb[starts[v]:ends[v]]
            if len(bkts):
                loads = cell[bkts]
                penalty = (np.maximum(loads + 1 - cap, 0) * 1000 + loads).sum(axis=0)
            else:
                penalty = np.zeros(N_SLABS)
            penalty = penalty + (qcap == 0) * 1e12
            q = int(np.argmin(penalty))
            src_slab[v] = q
            qcap[q] -= 1
            if len(bkts):
                np.add.at(cell, (bkts, q), 1)

    # repair: move one contributor out of each overflowing cell when possible
    eq_slab = src_slab[src]
    for _ in range(3):
        over = np.argwhere(cell > cap)
        if not len(over):
            break
        for bkt, q in over:
            if cell[bkt, q] <= cap:
                continue
            cand = np.unique(src[(edge_bucket == bkt) & (eq_slab == q)])
            moved = False
            for v in cand:
                bkts = sb[starts[v]:ends[v]]
                for q2 in range(N_SLABS):
                    if q2 == q:
                        continue
                    ub, mult = np.unique(bkts, return_counts=True)
                    if np.all(cell[ub, q2] + mult <= cap):
                        np.add.at(cell, (bkts, q), -1)
                        np.add.at(cell, (bkts, q2), 1)
                        src_slab[v] = q2
                        eq_slab = src_slab[src]
                        moved = True
                        break
                if moved:
                    break

    src_pos = np.zeros(N_NODES, dtype=np.int64)
    for c in range(N_CORES):
        lo = c * SHARD
        qs = src_slab[lo:lo + SHARD]
        if np.bincount(qs, minlength=N_SLABS).max() > QROWS:
            qs = np.repeat(np.arange(N_SLABS), QROWS)[:SHARD].astype(np.int8)
        fill = np.zeros(N_SLABS, dtype=np.int64)
        pos = np.empty(SHARD, dtype=np.int64)
        for i in range(SHARD):
            q = int(qs[i])
            pos[i] = q * QROWS + fill[q]
            fill[q] += 1
        src_pos[lo:lo + SHARD] = pos
    return dst_pos, src_pos


def _inspect(src, dst, dst_pos, src_pos):
    """Host inspector: per-core sorted edge data + shared static tile grid."""
    core = dst // SHARD
    e_blk = dst_pos[dst] >> 7
    e_rel = dst_pos[dst] & (P - 1)
    e_s8 = src // SHARD
    e_slab = src_pos[src] // QROWS
    e_gidx = e_s8 * QROWS + (src_pos[src] - e_slab * QROWS)
    gidx_of, dstrel_of = [], []
    counts = np.zeros((N_CORES, N_PASSES, N_SLABS, N_BLOCKS), dtype=np.int64)
    for c in range(N_CORES):
        m = core == c
        blk = e_blk[m]
        slab = e_slab[m]
        gidx = e_gidx[m]
        pss = PASS_OF_BLOCK[blk]
        diagkey = slab * DIAG_G + pss
        order = np.lexsort((gidx, blk, slab, diagkey))
        gidx_of.append(gidx[order])
        dstrel_of.append(e_rel[m][order])
        key = (pss * N_SLABS + slab) * N_BLOCKS + blk
        cnt = np.bincount(key, minlength=N_PASSES * N_SLABS * N_BLOCKS)
        counts[c] = cnt.reshape(N_PASSES, N_SLABS, N_BLOCKS)
    T = ((counts + P - 1) // P).max(axis=0)  # shared tile grid
    return gidx_of, dstrel_of, counts, T


def _make_schedule(T):
    """Diagonal (pass, slab) stream: work order tracks AllGather availability."""
    diag = sorted(((p, s) for p in range(N_PASSES) for s in range(N_SLABS)),
                  key=lambda ps: (ps[1] * DIAG_G + ps[0], ps[1]))
    tile_meta = []   # [p, s, b, start, stop]
    calls = []       # (s, j0, ct) one per (p, s) group
    groups = []      # (p, s) in stream order
    for (p, s) in diag:
        j0 = len(tile_meta)
        for b in range(PASS_BASE[p], PASS_BASE[p] + PASS_BLOCKS[p]):
            n = int(T[p, s, b])
            assert n >= 1, (p, s, b)
            for k in range(n):
                tile_meta.append([p, s, b, k == 0, k == n - 1])
        ct = len(tile_meta) - j0
        assert ct <= CTMAX, ct
        calls.append((s, j0, ct))
        groups.append((p, s))
    return tile_meta, calls, groups


def _per_core_streams(c, tile_meta, counts, gidx_of, dstrel_of):
    """This core's padded gather-idx + dst_rel streams matching the grid."""
    ntiles = len(tile_meta)
    idx_stream = np.zeros(ntiles * P, dtype=np.int16)
    dst_stream = np.full(ntiles * P, -1.0, dtype=np.float32)
    edge_ptr = 0
    j = 0
    while j < ntiles:
        p, s, b = tile_meta[j][:3]
        k = j
        while k < ntiles and tile_meta[k][:3] == [p, s, b]:
            k += 1
        nseg = int(counts[c, p, s, b])
        base = j * P
        idx_stream[base:base + nseg] = gidx_of[c][edge_ptr:edge_ptr + nseg]
        dst_stream[base:base + nseg] = dstrel_of[c][edge_ptr:edge_ptr + nseg]
        edge_ptr += nseg
        j = k
    assert edge_ptr == len(gidx_of[c])
    idx_wrapped = np.tile(idx_stream.reshape(-1, 16).T, (8, 1)).copy()
    dstv = dst_stream.reshape(ntiles, P).T.copy()
    return idx_wrapped, dstv


def _build(tile_meta, calls, groups):
    ntiles = len(tile_meta)
    nc = bacc.Bacc("TRN2", target_bir_lowering=False, num_swdge_queues=4)
    dt = mybir.dt

    featT = nc.declare_dram_parameter("featT", [IN_FEATS, SHARD_PAD], dt.bfloat16, isOutput=False)
    w_in = nc.declare_dram_parameter("w", [IN_FEATS, OUT_FEATS], dt.bfloat16, isOutput=False)
    biasb = nc.declare_dram_parameter("biasb", [P, OUT_FEATS], dt.float32, isOutput=False)
    idegw = nc.declare_dram_parameter("idegw", [P, N_BLOCKS], dt.float32, isOutput=False)
    odegw = nc.declare_dram_parameter("odegw", [P, N_BLOCKS], dt.float32, isOutput=False)
    iota_in = nc.declare_dram_parameter("iota", [P, P], dt.bfloat16, isOutput=False)
    idxs_in = nc.declare_dram_parameter("idxs", [P, ntiles * 8], dt.int16, isOutput=False)
    dstv_in = nc.declare_dram_parameter("dstv", [P, ntiles], dt.bfloat16, isOutput=False)
    out_d = nc.declare_dram_parameter("out", [SHARD_PAD, OUT_FEATS], dt.float32, isOutput=True)

    tableL = nc.dram_tensor("tableL", [SHARD_PAD, PAIR], dt.bfloat16)
    tableQ = [nc.dram_tensor(f"tableQ{q}", [SLAB_ROWS, PAIR], dt.bfloat16,
                             addr_space="Shared") for q in range(N_SLABS)]

    with tile.TileContext(nc) as tc:
        with tc.tile_pool(name="const", bufs=1) as constp, \
             tc.tile_pool(name="gp", bufs=10) as gp, \
             tc.tile_pool(name="dmp", bufs=8) as dmp, \
             tc.tile_pool(name="sp", bufs=8) as sps, \
             tc.tile_pool(name="outp", bufs=4) as outp:

            # ---- constants ----
            w_sb = constp.tile([P, 2, OUT_FEATS], dt.bfloat16)
            for k in range(2):
                nc.sync.dma_start(out=w_sb[:, k, :], in_=w_in[k * P:(k + 1) * P, :])
            bias_sb = constp.tile([P, 1, OUT_FEATS], dt.float32)
            nc.sync.dma_start(out=bias_sb[:, 0, :], in_=biasb[:])
            iota4 = constp.tile([P, SW, P], dt.bfloat16)
            for k in range(SW):
                nc.sync.dma_start(out=iota4[:, k, :], in_=iota_in[:])
            dstv_sb = constp.tile([P, ntiles, 1], dt.bfloat16)
            nc.sync.dma_start(out=dstv_sb[:, :, 0], in_=dstv_in[:])
            idx_sb = constp.tile([P, ntiles * 8], dt.int16)
            nc.sync.dma_start(out=idx_sb[:], in_=idxs_in[:])
            # SBUF accumulator for partial block sums (hi|lo in fp32)
            outacc = constp.tile([P, N_BLOCKS, PAIR], dt.float32)
            nc.vector.memset(outacc[:], 0.0)

            # ---- phase 1: table build (pools scoped to free SBUF/PSUM) ----
            with tc.tile_pool(name="ft", bufs=1) as ftp, \
                 tc.tile_pool(name="ph1", bufs=4) as ph1, \
                 tc.tile_pool(name="ph1ps", bufs=4, space="PSUM") as ph1ps:

                ideg_sb = ph1.tile([P, N_BLOCKS], dt.float32, tag="deg")
                odeg_sb = ph1.tile([P, N_BLOCKS], dt.float32, tag="deg")
                nc.sync.dma_start(out=ideg_sb[:], in_=idegw[:])
                nc.sync.dma_start(out=odeg_sb[:], in_=odegw[:])
                scale_sb = constp.tile([P, N_BLOCKS], dt.float32)
                nc.vector.tensor_scalar_max(ideg_sb[:], ideg_sb[:], 1.0)
                nc.vector.tensor_scalar_max(odeg_sb[:], odeg_sb[:], 1.0)
                nc.vector.tensor_mul(out=scale_sb[:], in0=ideg_sb[:], in1=odeg_sb[:])
                nc.scalar.activation(out=scale_sb[:], in_=scale_sb[:],
                                     func=mybir.ActivationFunctionType.Sqrt)
                nc.vector.reciprocal(out=scale_sb[:], in_=scale_sb[:])

                # featT in chunks (2 k-chunks x 8 column chunks)
                FCH = [13] * 7 + [7]
                FBASE = [0, 13, 26, 39, 52, 65, 78, 91]
                ft_sb = {}
                for fc in range(8):
                    for k in range(2):
                        t_ = ftp.tile([P, FCH[fc] * P], dt.bfloat16, tag=f"ft{k}", bufs=2)
                        nc.sync.dma_start(
                            out=t_[:],
                            in_=featT[k * P:(k + 1) * P,
                                      FBASE[fc] * P:(FBASE[fc] + FCH[fc]) * P])
                        ft_sb[(fc, k)] = t_

                for t in range(N_BLOCKS):
                    fc = min(t // 13, 7)
                    tc_rel = t - FBASE[fc]
                    hp = ph1ps.tile([P, OUT_FEATS], dt.float32, tag="hps")
                    for k in range(2):
                        nc.tensor.matmul(
                            out=hp[:],
                            lhsT=ft_sb[(fc, k)][:, tc_rel * P:(tc_rel + 1) * P],
                            rhs=w_sb[:, k, :],
                            start=(k == 0), stop=(k == 1),
                        )
                    h = ph1.tile([P, OUT_FEATS], dt.float32, tag="h")
                    nc.vector.tensor_copy(out=h[:], in_=hp[:])
                    m1 = ph1.tile([P, 8], dt.float32, tag="m1")
                    nc.vector.max(m1[:], h[:])
                    hneg = ph1.tile([P, OUT_FEATS], dt.float32, tag="hneg")
                    nc.vector.match_replace(out=hneg[:], in_to_replace=m1[:],
                                            in_values=h[:], imm_value=NEG_INF)
                    m2 = ph1.tile([P, 8], dt.float32, tag="m2")
                    nc.vector.max(m2[:], hneg[:])
                    # hm = (h >= thr) * h  in one fused op
                    hm = ph1.tile([P, OUT_FEATS], dt.float32, tag="mask")
                    nc.vector.scalar_tensor_tensor(
                        out=hm[:], in0=h[:], scalar=m2[:, 7:8], in1=h[:],
                        op0=mybir.AluOpType.is_ge, op1=mybir.AluOpType.mult)
                    ttile = ph1.tile([P, PAIR], dt.bfloat16, tag="ttile")
                    hi32 = ph1.tile([P, OUT_FEATS], dt.float32, tag="hi32")
                    # hi = bf16(hm * scale) via ACT's fused input scale
                    nc.scalar.activation(out=ttile[:, 0:OUT_FEATS], in_=hm[:],
                                         func=mybir.ActivationFunctionType.Copy,
                                         scale=scale_sb[:, t:t + 1])
                    nc.scalar.activation(out=hi32[:], in_=ttile[:, 0:OUT_FEATS],
                                         func=mybir.ActivationFunctionType.Copy)
                    # lo = bf16(hm * scale - hi32) in one fused op
                    nc.vector.scalar_tensor_tensor(
                        out=ttile[:, OUT_FEATS:PAIR], in0=hm[:],
                        scalar=scale_sb[:, t:t + 1], in1=hi32[:],
                        op0=mybir.AluOpType.mult,
                        op1=mybir.AluOpType.subtract)
                    nc.sync.dma_start(out=tableL[t * P:(t + 1) * P, :], in_=ttile[:])

            # ---- allgather table, one collective per slab (8 chunks) so
            # phase-2 gathers start after the first chunk ----
            for q in range(N_SLABS):
                nc.gpsimd.collective_compute(
                    "AllGather",
                    mybir.AluOpType.bypass,
                    replica_groups=[list(range(N_CORES))],
                    ins=[tableL[q * QROWS:(q + 1) * QROWS, :]],
                    outs=[tableQ[q][:]],
                )

            # ---- phase 2: edge aggregation, diagonal (pass, slab) order ----
            phase2_stack = __import__("contextlib").ExitStack()
            accp = phase2_stack.enter_context(
                tc.tile_pool(name="accp", bufs=2, space="PSUM"))
            slab_seen = {}
            for gi, ((p, s), (s_, j0, ct)) in enumerate(zip(groups, calls)):
                nblk = PASS_BLOCKS[p]
                g = gp.tile([P, CTMAX, PAIR], dt.bfloat16, tag="g")
                nc.gpsimd.dma_gather(
                    out_ap=g[:, :ct, :],
                    in_ap=tableQ[s][:],
                    idxs_ap=idx_sb[:, j0 * 8:(j0 + ct) * 8],
                    num_idxs=ct * P,
                    num_idxs_reg=ct * P,
                    elem_size=PAIR,
                    single_packet=False,
                    queue_num=s % 4,
                )
                # one-hot S tiles: scalar engine materializes dstv (broadcast
                # copy), DVE compares against a packed iota (2-byte mode)
                s_tiles = []
                for t0 in range(0, ct, SW):
                    jn = min(SW, ct - t0)
                    dm = dmp.tile([P, SW, P], dt.bfloat16, tag="dm")
                    nc.scalar.copy(
                        out=dm[:, :jn, :],
                        in_=dstv_sb[:, j0 + t0:j0 + t0 + jn, :].to_broadcast(
                            [P, jn, P]))
                    s4 = sps.tile([P, SW, P], dt.bfloat16, tag="s")
                    nc.vector.tensor_tensor(
                        out=s4[:, :jn, :], in0=dm[:, :jn, :],
                        in1=iota4[:, :jn, :],
                        op=mybir.AluOpType.is_equal)
                    for t in range(jn):
                        s_tiles.append((s4, t))
                acc = accp.tile([P, 4, 512], dt.float32, tag="acc")
                for t in range(ct):
                    p_, s2, b, st, sp_ = tile_meta[j0 + t]
                    b_rel = b - PASS_BASE[p]
                    s4_t, s4_i = s_tiles[t]
                    nc.tensor.matmul(
                        out=acc[:, b_rel, 0:PAIR],
                        lhsT=s4_t[:, s4_i, :],
                        rhs=g[:, t, :],
                        start=bool(st), stop=bool(sp_),
                        skip_group_check=True,
                    )
                # evict group partials into the SBUF accumulator
                B = PASS_BASE[p]
                nc.vector.tensor_add(
                    out=outacc[:, B:B + nblk, :],
                    in0=outacc[:, B:B + nblk, :],
                    in1=acc[:, :nblk, 0:PAIR])
                seen = slab_seen.get(p, 0) + 1
                slab_seen[p] = seen
                if seen == N_SLABS:
                    # final fold: hi + lo + bias, then DMA out
                    hs = outp.tile([P, 4, OUT_FEATS], dt.float32, tag="hs")
                    nc.vector.tensor_add(
                        out=hs[:, :nblk, :],
                        in0=outacc[:, B:B + nblk, 0:OUT_FEATS],
                        in1=outacc[:, B:B + nblk, OUT_FEATS:PAIR])
                    nc.vector.tensor_add(
                        out=hs[:, :nblk, :],
                        in0=hs[:, :nblk, :],
                        in1=bias_sb[:].to_broadcast([P, nblk, OUT_FEATS]))
                    for b_rel in range(nblk):
                        b = B + b_rel
                        nc.sync.dma_start(out=out_d[b * P:(b + 1) * P, :],
                                          in_=hs[:, b_rel, :])
            phase2_stack.close()

    nc.finalize()
    return nc


def kernel(feat, weight, bias, src, dst):
    feat = np.asarray(feat, dtype=np.float32)
    weight = np.asarray(weight, dtype=np.float32)
    bias = np.asarray(bias, dtype=np.float32)
    src = np.asarray(src)
    dst = np.asarray(dst)

    src64 = src.astype(np.int64)
    dst64 = dst.astype(np.int64)
    in_deg = np.bincount(dst64, minlength=N_NODES).astype(np.float32)
    out_deg = np.bincount(src64, minlength=N_NODES).astype(np.float32)
    dst_pos, src_pos = _balance_perms(src64, dst64,
                                      np.bincount(dst64, minlength=N_NODES),
                                      np.bincount(src64, minlength=N_NODES))
    gidx_of, dstrel_of, counts, T = _inspect(src64, dst64, dst_pos, src_pos)
    tile_meta, calls, groups = _make_schedule(T)

    ft = feat.T  # [256, 100000]
    iota = np.tile(np.arange(P, dtype=np.float32), (P, 1)).astype(ml_dtypes.bfloat16)

    in_maps = []
    for c in range(N_CORES):
        lo, hi = c * SHARD, (c + 1) * SHARD
        featT_c = np.zeros((IN_FEATS, SHARD_PAD), dtype=np.float32)
        featT_c[:, src_pos[lo:hi]] = ft[:, lo:hi]
        ideg_c = np.ones(SHARD_PAD, dtype=np.float32)
        odeg_c = np.ones(SHARD_PAD, dtype=np.float32)
        ideg_c[src_pos[lo:hi]] = in_deg[lo:hi]
        odeg_c[src_pos[lo:hi]] = out_deg[lo:hi]
        idx_wrapped, dstv = _per_core_streams(c, tile_meta, counts,
                                              gidx_of, dstrel_of)
        in_maps.append({
            "featT": featT_c.astype(ml_dtypes.bfloat16),
            "w": weight.astype(ml_dtypes.bfloat16),
            "biasb": np.tile(bias[None, :], (P, 1)).astype(np.float32),
            "idegw": ideg_c.reshape(N_BLOCKS, P).T.copy(),
            "odegw": odeg_c.reshape(N_BLOCKS, P).T.copy(),
            "iota": iota,
            "idxs": idx_wrapped,
            "dstv": dstv.astype(ml_dtypes.bfloat16),
        })

    nc = _build(tile_meta, calls, groups)
    res = run_bass_kernel_spmd(nc, in_maps, list(range(N_CORES)),
                               trace=bool(os.environ.get("KERNEL_TRACE")))
    if os.environ.get("KERNEL_TRACE"):
        print(f"HW exec time: {res.exec_time_ns} ns")
    out = np.empty((N_NODES, OUT_FEATS), dtype=np.float32)
    for c in range(N_CORES):
        lo, hi = c * SHARD, (c + 1) * SHARD
        out[lo:hi] = res.results[c]["out"][dst_pos[lo:hi]]
    return out


# revision 10
# speedup vs baseline: 1.0102x; 1.0102x over previous
"""MaxK-GCN conv on 8 Trainium2 NeuronCores.

Pipeline (per core c, SPMD over 8 cores; nodes sharded 8 x 12500):
  host: load-balancing permutations — dst nodes LPT-packed into 98 blocks
        (~2045 in-edges each), src nodes greedy-packed into 8 slab-quarters
        so every (slab, block) bucket holds <=256 edges (2 gather tiles).
  phase 1: h = featT_c.T @ W (PE, bf16), top-16-of-64 threshold mask (DVE
           max8 + match_replace), scale by (max(out_deg,1)*max(in_deg,1))^-0.5,
           split each fp32 row into a [hi|lo] bf16 pair -> local table shard
           [12544, 128] bf16 (hi+lo reconstructs fp32 to ~2^-17).
  AllGather the table in 8 slab chunks so phase-2 gathers start after the
  first chunk (~45us) instead of after the full table.
  phase 2: per (pass, slab) group in slab-diagonal order (work tracks
           AllGather availability): dma_gather src rows (SWDGE), one-hot S
           tiles (scalar-engine broadcast-copy + DVE is_eq, keeping DVE in
           2-byte packed mode), matmul S^T @ G per 128-dst block in PSUM,
           evict per-group partial sums into an SBUF accumulator (decouples
           PSUM from pass completion), fold hi+lo + bias after the last
           slab, DMA out.

Edge bookkeeping (permutations, sort, padding, degree counts) is host-side
index metadata; all floating-point math runs on device.
"""
import sys
import os

sys.path.insert(0, "/opt/trn_rl_repo")

import numpy as np
import ml_dtypes
import concourse.bacc as bacc
import concourse.mybir as mybir
import concourse.tile as tile
from concourse.bass_utils import run_bass_kernel_spmd

P = 128
N_NODES = 100000
IN_FEATS = 256
OUT_FEATS = 64
N_CORES = 8
SHARD = N_NODES // N_CORES          # 12500 real nodes per core
SHARD_PAD = 12544                   # 98 * 128
N_BLOCKS = SHARD_PAD // P           # 98
N_SLABS = 8
QROWS = SHARD_PAD // N_SLABS        # 1568 rows per shard-slab
SLAB_ROWS = N_CORES * QROWS         # 12544 rows per slab-table
PASS_BLOCKS = [4] * 24 + [2]        # blocks per PSUM pass
N_PASSES = len(PASS_BLOCKS)
TABLE_ROWS = N_CORES * SHARD_PAD    # 100352
PAIR = 2 * OUT_FEATS                # 128 bf16 per table row (hi|lo)
CTMAX = 16                          # max tiles per (pass, slab) gather call
DIAG_G = 10                         # diagonal lag: calls per AllGather gap
SW = 4                              # one-hot build batch width
NEG_INF = -3.0e38

PASS_OF_BLOCK = np.repeat(np.arange(N_PASSES), PASS_BLOCKS)
PASS_BASE = np.cumsum([0] + PASS_BLOCKS[:-1])


def _balance_perms(src, dst, in_deg, out_deg):
    """Load-balancing layout: per-core dst->block LPT + src->slab greedy.

    Returns (dst_pos, src_pos): position of each node within its core's
    padded shard, for the output rows (dst_pos) and table rows (src_pos).
    """
    import heapq

    dst_pos = np.zeros(N_NODES, dtype=np.int64)
    for c in range(N_CORES):
        lo = c * SHARD
        deg = in_deg[lo:lo + SHARD]
        order = np.argsort(-deg, kind="stable")
        loads = np.zeros(N_BLOCKS, dtype=np.int64)
        slots = np.zeros(N_BLOCKS, dtype=np.int64)
        heap = [(0, b) for b in range(N_BLOCKS)]
        heapq.heapify(heap)
        pos = np.empty(SHARD, dtype=np.int64)
        for v in order:
            while True:
                _, b = heapq.heappop(heap)
                if slots[b] < P:
                    break
            pos[v] = b * P + slots[b]
            slots[b] += 1
            loads[b] += deg[v]
            if slots[b] < P:
                heapq.heappush(heap, (loads[b], b))
        dst_pos[lo:lo + SHARD] = pos

    # src->slab greedy: balance each consumer bucket (dst core, block)
    # across the 8 slab-tables; cells <= 256 keep buckets at 2 tiles.
    edge_bucket = ((dst // SHARD) * N_BLOCKS + (dst_pos[dst] // P)).astype(np.int64)
    nbkt = N_CORES * N_BLOCKS
    cell = np.zeros((nbkt, N_SLABS), dtype=np.int32)
    cap = 2 * P
    src_slab = np.zeros(N_NODES, dtype=np.int8)
    order_e = np.argsort(src, kind="stable")
    sb = edge_bucket[order_e]
    s_sorted = src[order_e]
    starts = np.searchsorted(s_sorted, np.arange(N_NODES))
    ends = np.searchsorted(s_sorted, np.arange(N_NODES) + 1)
    for c in range(N_CORES):
        lo = c * SHARD
        node_order = np.argsort(-out_deg[lo:lo + SHARD], kind="stable") + lo
        qcap = np.full(N_SLABS, QROWS, dtype=np.int64)
        for v in node_order:
            bkts = sb[starts[v]:ends[v]]
            if len(bkts):
                loads = cell[bkts]
                penalty = (np.maximum(loads + 1 - cap, 0) * 1000 + loads).sum(axis=0)
            else:
                penalty = np.zeros(N_SLABS)
            penalty = penalty + (qcap == 0) * 1e12
            q = int(np.argmin(penalty))
            src_slab[v] = q
            qcap[q] -= 1
            if len(bkts):
                np.add.at(cell, (bkts, q), 1)

    # repair: move one contributor out of each overflowing cell when possible
    eq_slab = src_slab[src]
    for _ in range(3):
        over = np.argwhere(cell > cap)
        if not len(over):
            break
        for bkt, q in over:
            if cell[bkt, q] <= cap:
                continue
            cand = np.unique(src[(edge_bucket == bkt) & (eq_slab == q)])
            moved = False
            for v in cand:
                bkts = sb[starts[v]:ends[v]]
                for q2 in range(N_SLABS):
                    if q2 == q:
                        continue
                    ub, mult = np.unique(bkts, return_counts=True)
                    if np.all(cell[ub, q2] + mult <= cap):
                        np.add.at(cell, (bkts, q), -1)
                        np.add.at(cell, (bkts, q2), 1)
                        src_slab[v] = q2
                        eq_slab = src_slab[src]
                        moved = True
                        break
                if moved:
                    break

    src_pos = np.zeros(N_NODES, dtype=np.int64)
    for c in range(N_CORES):
        lo = c * SHARD
        qs = src_slab[lo:lo + SHARD]
        if np.bincount(qs, minlength=N_SLABS).max() > QROWS:
            qs = np.repeat(np.arange(N_SLABS), QROWS)[:SHARD].astype(np.int8)
        fill = np.zeros(N_SLABS, dtype=np.int64)
        pos = np.empty(SHARD, dtype=np.int64)
        for i in range(SHARD):
            q = int(qs[i])
            pos[i] = q * QROWS + fill[q]
            fill[q] += 1
        src_pos[lo:lo + SHARD] = pos
    return dst_pos, src_pos


def _inspect(src, dst, dst_pos, src_pos):
    """Host inspector: per-core sorted edge data + shared static tile grid."""
    core = dst // SHARD
    e_blk = dst_pos[dst] >> 7
    e_rel = dst_pos[dst] & (P - 1)
    e_s8 = src // SHARD
    e_slab = src_pos[src] // QROWS
    e_gidx = e_s8 * QROWS + (src_pos[src] - e_slab * QROWS)
    gidx_of, dstrel_of = [], []
    counts = np.zeros((N_CORES, N_PASSES, N_SLABS, N_BLOCKS), dtype=np.int64)
    for c in range(N_CORES):
        m = core == c
        blk = e_blk[m]
        slab = e_slab[m]
        gidx = e_gidx[m]
        pss = PASS_OF_BLOCK[blk]
        diagkey = slab * DIAG_G + pss
        order = np.lexsort((gidx, blk, slab, diagkey))
        gidx_of.append(gidx[order])
        dstrel_of.append(e_rel[m][order])
        key = (pss * N_SLABS + slab) * N_BLOCKS + blk
        cnt = np.bincount(key, minlength=N_PASSES * N_SLABS * N_BLOCKS)
        counts[c] = cnt.reshape(N_PASSES, N_SLABS, N_BLOCKS)
    T = ((counts + P - 1) // P).max(axis=0)  # shared tile grid
    return gidx_of, dstrel_of, counts, T


def _make_schedule(T):
    """Diagonal (pass, slab) stream: work order tracks AllGather availability."""
    diag = sorted(((p, s) for p in range(N_PASSES) for s in range(N_SLABS)),
                  key=lambda ps: (ps[1] * DIAG_G + ps[0], ps[1]))
    tile_meta = []   # [p, s, b, start, stop]
    calls = []       # (s, j0, ct) one per (p, s) group
    groups = []      # (p, s) in stream order
    for (p, s) in diag:
        j0 = len(tile_meta)
        for b in range(PASS_BASE[p], PASS_BASE[p] + PASS_BLOCKS[p]):
            n = int(T[p, s, b])
            assert n >= 1, (p, s, b)
            for k in range(n):
                tile_meta.append([p, s, b, k == 0, k == n - 1])
        ct = len(tile_meta) - j0
        assert ct <= CTMAX, ct
        calls.append((s, j0, ct))
        groups.append((p, s))
    return tile_meta, calls, groups


def _per_core_streams(c, tile_meta, counts, gidx_of, dstrel_of):
    """This core's padded gather-idx + dst_rel streams matching the grid."""
    ntiles = len(tile_meta)
    idx_stream = np.zeros(ntiles * P, dtype=np.int16)
    dst_stream = np.full(ntiles * P, -1.0, dtype=np.float32)
    edge_ptr = 0
    j = 0
    while j < ntiles:
        p, s, b = tile_meta[j][:3]
        k = j
        while k < ntiles and tile_meta[k][:3] == [p, s, b]:
            k += 1
        nseg = int(counts[c, p, s, b])
        base = j * P
        idx_stream[base:base + nseg] = gidx_of[c][edge_ptr:edge_ptr + nseg]
        dst_stream[base:base + nseg] = dstrel_of[c][edge_ptr:edge_ptr + nseg]
        edge_ptr += nseg
        j = k
    assert edge_ptr == len(gidx_of[c])
    idx_wrapped = np.tile(idx_stream.reshape(-1, 16).T, (8, 1)).copy()
    dstv = dst_stream.reshape(ntiles, P).T.copy()
    return idx_wrapped, dstv


def _build(tile_meta, calls, groups):
    ntiles = len(tile_meta)
    nc = bacc.Bacc("TRN2", target_bir_lowering=False, num_swdge_queues=4)
    dt = mybir.dt

    featT = nc.declare_dram_parameter("featT", [IN_FEATS, SHARD_PAD], dt.float32, isOutput=False)
    w_in = nc.declare_dram_parameter("w", [IN_FEATS, OUT_FEATS], dt.float32, isOutput=False)
    biasb = nc.declare_dram_parameter("biasb", [P, OUT_FEATS], dt.float32, isOutput=False)
    idegw = nc.declare_dram_parameter("idegw", [P, N_BLOCKS], dt.float32, isOutput=False)
    odegw = nc.declare_dram_parameter("odegw", [P, N_BLOCKS], dt.float32, isOutput=False)
    iota_in = nc.declare_dram_parameter("iota", [P, P], dt.bfloat16, isOutput=False)
    idxs_in = nc.declare_dram_parameter("idxs", [P, ntiles * 8], dt.int16, isOutput=False)
    dstv_in = nc.declare_dram_parameter("dstv", [P, ntiles], dt.bfloat16, isOutput=False)
    out_d = nc.declare_dram_parameter("out", [SHARD_PAD, OUT_FEATS], dt.float32, isOutput=True)

    tableL = nc.dram_tensor("tableL", [SHARD_PAD, PAIR], dt.bfloat16)
    tableQ = [nc.dram_tensor(f"tableQ{q}", [SLAB_ROWS, PAIR], dt.bfloat16,
                             addr_space="Shared") for q in range(N_SLABS)]

    with tile.TileContext(nc) as tc:
        with tc.tile_pool(name="const", bufs=1) as constp, \
             tc.tile_pool(name="gp", bufs=10) as gp, \
             tc.tile_pool(name="dmp", bufs=8) as dmp, \
             tc.tile_pool(name="sp", bufs=8) as sps, \
             tc.tile_pool(name="outp", bufs=4) as outp:

            # ---- constants ----
            w_sb = constp.tile([P, 2, OUT_FEATS], dt.float32)
            for k in range(2):
                nc.sync.dma_start(out=w_sb[:, k, :], in_=w_in[k * P:(k + 1) * P, :])
            bias_sb = constp.tile([P, 1, OUT_FEATS], dt.float32)
            nc.sync.dma_start(out=bias_sb[:, 0, :], in_=biasb[:])
            iota4 = constp.tile([P, SW, P], dt.bfloat16)
            for k in range(SW):
                nc.sync.dma_start(out=iota4[:, k, :], in_=iota_in[:])
            dstv_sb = constp.tile([P, ntiles, 1], dt.bfloat16)
            nc.sync.dma_start(out=dstv_sb[:, :, 0], in_=dstv_in[:])
            idx_sb = constp.tile([P, ntiles * 8], dt.int16)
            nc.sync.dma_start(out=idx_sb[:], in_=idxs_in[:])

            # ---- phase 1: table build (pools scoped to free SBUF/PSUM) ----
            with tc.tile_pool(name="ft", bufs=1) as ftp, \
                 tc.tile_pool(name="ph1", bufs=4) as ph1, \
                 tc.tile_pool(name="ph1ps", bufs=4, space="PSUM") as ph1ps:

                ideg_sb = ph1.tile([P, N_BLOCKS], dt.float32, tag="deg")
                odeg_sb = ph1.tile([P, N_BLOCKS], dt.float32, tag="deg")
                nc.sync.dma_start(out=ideg_sb[:], in_=idegw[:])
                nc.sync.dma_start(out=odeg_sb[:], in_=odegw[:])
                scale_sb = constp.tile([P, N_BLOCKS], dt.float32)
                nc.vector.tensor_scalar_max(ideg_sb[:], ideg_sb[:], 1.0)
                nc.vector.tensor_scalar_max(odeg_sb[:], odeg_sb[:], 1.0)
                nc.vector.tensor_mul(out=scale_sb[:], in0=ideg_sb[:], in1=odeg_sb[:])
                nc.scalar.activation(out=scale_sb[:], in_=scale_sb[:],
                                     func=mybir.ActivationFunctionType.Sqrt)
                nc.vector.reciprocal(out=scale_sb[:], in_=scale_sb[:])

                # featT in chunks (2 k-chunks x 8 column chunks)
                FCH = [13] * 7 + [7]
                FBASE = [0, 13, 26, 39, 52, 65, 78, 91]
                ft_sb = {}
                for fc in range(8):
                    for k in range(2):
                        t_ = ftp.tile([P, FCH[fc] * P], dt.float32, tag=f"ft{k}", bufs=2)
                        nc.sync.dma_start(
                            out=t_[:],
                            in_=featT[k * P:(k + 1) * P,
                                      FBASE[fc] * P:(FBASE[fc] + FCH[fc]) * P])
                        ft_sb[(fc, k)] = t_

                for t in range(N_BLOCKS):
                    fc = min(t // 13, 7)
                    tc_rel = t - FBASE[fc]
                    hp = ph1ps.tile([P, OUT_FEATS], dt.float32, tag="hps")
                    for k in range(2):
                        nc.tensor.matmul(
                            out=hp[:],
                            lhsT=ft_sb[(fc, k)][:, tc_rel * P:(tc_rel + 1) * P],
                            rhs=w_sb[:, k, :],
                            start=(k == 0), stop=(k == 1),
                        )
                    h = ph1.tile([P, OUT_FEATS], dt.float32, tag="h")
                    nc.vector.tensor_copy(out=h[:], in_=hp[:])
                    m1 = ph1.tile([P, 8], dt.float32, tag="m1")
                    nc.vector.max(m1[:], h[:])
                    hneg = ph1.tile([P, OUT_FEATS], dt.float32, tag="hneg")
                    nc.vector.match_replace(out=hneg[:], in_to_replace=m1[:],
                                            in_values=h[:], imm_value=NEG_INF)
                    m2 = ph1.tile([P, 8], dt.float32, tag="m2")
                    nc.vector.max(m2[:], hneg[:])
                    # hm = (h >= thr) * h  in one fused op
                    hm = ph1.tile([P, OUT_FEATS], dt.float32, tag="mask")
                    nc.vector.scalar_tensor_tensor(
                        out=hm[:], in0=h[:], scalar=m2[:, 7:8], in1=h[:],
                        op0=mybir.AluOpType.is_ge, op1=mybir.AluOpType.mult)
                    ttile = ph1.tile([P, PAIR], dt.bfloat16, tag="ttile")
                    hi32 = ph1.tile([P, OUT_FEATS], dt.float32, tag="hi32")
                    # hi = bf16(hm * scale) via ACT's fused input scale
                    nc.scalar.activation(out=ttile[:, 0:OUT_FEATS], in_=hm[:],
                                         func=mybir.ActivationFunctionType.Copy,
                                         scale=scale_sb[:, t:t + 1])
                    nc.scalar.activation(out=hi32[:], in_=ttile[:, 0:OUT_FEATS],
                                         func=mybir.ActivationFunctionType.Copy)
                    # lo = bf16(hm * scale - hi32) in one fused op
                    nc.vector.scalar_tensor_tensor(
                        out=ttile[:, OUT_FEATS:PAIR], in0=hm[:],
                        scalar=scale_sb[:, t:t + 1], in1=hi32[:],
                        op0=mybir.AluOpType.mult,
                        op1=mybir.AluOpType.subtract)
                    nc.sync.dma_start(out=tableL[t * P:(t + 1) * P, :], in_=ttile[:])

            # ---- allgather table, one collective per slab (8 chunks) so
            # phase-2 gathers start after the first chunk ----
            for q in range(N_SLABS):
                nc.gpsimd.collective_compute(
                    "AllGather",
                    mybir.AluOpType.bypass,
                    replica_groups=[list(range(N_CORES))],
                    ins=[tableL[q * QROWS:(q + 1) * QROWS, :]],
                    outs=[tableQ[q][:]],
                )

            # ---- phase 2: edge aggregation, diagonal (pass, slab) order ----
            phase2_stack = __import__("contextlib").ExitStack()
            accp = phase2_stack.enter_context(
                tc.tile_pool(name="accp", bufs=2, space="PSUM"))
            oap = phase2_stack.enter_context(
                tc.tile_pool(name="oap", bufs=N_PASSES))
            slab_seen = {}
            pass_acc = {}
            for gi, ((p, s), (s_, j0, ct)) in enumerate(zip(groups, calls)):
                nblk = PASS_BLOCKS[p]
                g = gp.tile([P, CTMAX, PAIR], dt.bfloat16, tag="g")
                nc.gpsimd.dma_gather(
                    out_ap=g[:, :ct, :],
                    in_ap=tableQ[s][:],
                    idxs_ap=idx_sb[:, j0 * 8:(j0 + ct) * 8],
                    num_idxs=ct * P,
                    num_idxs_reg=ct * P,
                    elem_size=PAIR,
                    single_packet=False,
                    queue_num=s % 4,
                )
                # one-hot S tiles: scalar engine materializes dstv (broadcast
                # copy), DVE compares against a packed iota (2-byte mode)
                s_tiles = []
                for t0 in range(0, ct, SW):
                    jn = min(SW, ct - t0)
                    dm = dmp.tile([P, SW, P], dt.bfloat16, tag="dm")
                    nc.scalar.copy(
                        out=dm[:, :jn, :],
                        in_=dstv_sb[:, j0 + t0:j0 + t0 + jn, :].to_broadcast(
                            [P, jn, P]))
                    s4 = sps.tile([P, SW, P], dt.bfloat16, tag="s")
                    nc.vector.tensor_tensor(
                        out=s4[:, :jn, :], in0=dm[:, :jn, :],
                        in1=iota4[:, :jn, :],
                        op=mybir.AluOpType.is_equal)
                    for t in range(jn):
                        s_tiles.append((s4, t))
                acc = accp.tile([P, 4, 512], dt.float32, tag="acc")
                for t in range(ct):
                    p_, s2, b, st, sp_ = tile_meta[j0 + t]
                    b_rel = b - PASS_BASE[p]
                    s4_t, s4_i = s_tiles[t]
                    nc.tensor.matmul(
                        out=acc[:, b_rel, 0:PAIR],
                        lhsT=s4_t[:, s4_i, :],
                        rhs=g[:, t, :],
                        start=bool(st), stop=bool(sp_),
                        skip_group_check=True,
                    )
                # evict group partials into this pass's SBUF accumulator
                B = PASS_BASE[p]
                if p not in pass_acc:
                    pa = oap.tile([P, 4, PAIR], dt.float32, tag="pa")
                    nc.vector.tensor_copy(out=pa[:, :nblk, :],
                                          in_=acc[:, :nblk, 0:PAIR])
                    pass_acc[p] = pa
                else:
                    pa = pass_acc[p]
                    nc.vector.tensor_add(
                        out=pa[:, :nblk, :],
                        in0=pa[:, :nblk, :],
                        in1=acc[:, :nblk, 0:PAIR])
                seen = slab_seen.get(p, 0) + 1
                slab_seen[p] = seen
                if seen == N_SLABS:
                    # final fold: hi + lo + bias, then DMA out
                    hs = outp.tile([P, 4, OUT_FEATS], dt.float32, tag="hs")
                    nc.vector.tensor_add(
                        out=hs[:, :nblk, :],
                        in0=pa[:, :nblk, 0:OUT_FEATS],
                        in1=pa[:, :nblk, OUT_FEATS:PAIR])
                    nc.vector.tensor_add(
                        out=hs[:, :nblk, :],
                        in0=hs[:, :nblk, :],
                        in1=bias_sb[:].to_broadcast([P, nblk, OUT_FEATS]))
                    for b_rel in range(nblk):
                        b = B + b_rel
                        nc.sync.dma_start(out=out_d[b * P:(b + 1) * P, :],
                                          in_=hs[:, b_rel, :])
            phase2_stack.close()

    nc.finalize()
    return nc


def kernel(feat, weight, bias, src, dst):
    feat = np.asarray(feat, dtype=np.float32)
    weight = np.asarray(weight, dtype=np.float32)
    bias = np.asarray(bias, dtype=np.float32)
    src = np.asarray(src)
    dst = np.asarray(dst)

    src64 = src.astype(np.int64)
    dst64 = dst.astype(np.int64)
    in_deg = np.bincount(dst64, minlength=N_NODES).astype(np.float32)
    out_deg = np.bincount(src64, minlength=N_NODES).astype(np.float32)
    dst_pos, src_pos = _balance_perms(src64, dst64,
                                      np.bincount(dst64, minlength=N_NODES),
                                      np.bincount(src64, minlength=N_NODES))
    gidx_of, dstrel_of, counts, T = _inspect(src64, dst64, dst_pos, src_pos)
    tile_meta, calls, groups = _make_schedule(T)

    ft = feat.T  # [256, 100000]
    iota = np.tile(np.arange(P, dtype=np.float32), (P, 1)).astype(ml_dtypes.bfloat16)

    in_maps = []
    for c in range(N_CORES):
        lo, hi = c * SHARD, (c + 1) * SHARD
        featT_c = np.zeros((IN_FEATS, SHARD_PAD), dtype=np.float32)
        featT_c[:, src_pos[lo:hi]] = ft[:, lo:hi]
        ideg_c = np.ones(SHARD_PAD, dtype=np.float32)
        odeg_c = np.ones(SHARD_PAD, dtype=np.float32)
        ideg_c[src_pos[lo:hi]] = in_deg[lo:hi]
        odeg_c[src_pos[lo:hi]] = out_deg[lo:hi]
        idx_wrapped, dstv = _per_core_streams(c, tile_meta, counts,
                                              gidx_of, dstrel_of)
        in_maps.append({
            "featT": featT_c,
            "w": weight,
            "biasb": np.tile(bias[None, :], (P, 1)).astype(np.float32),
            "idegw": ideg_c.reshape(N_BLOCKS, P).T.copy(),
            "odegw": odeg_c.reshape(N_BLOCKS, P).T.copy(),
            "iota": iota,
            "idxs": idx_wrapped,
            "dstv": dstv.astype(ml_dtypes.bfloat16),
        })

    nc = _build(tile_meta, calls, groups)
    res = run_bass_kernel_spmd(nc, in_maps, list(range(N_CORES)),
                               trace=bool(os.environ.get("KERNEL_TRACE")))
    if os.environ.get("KERNEL_TRACE"):
        print(f"HW exec time: {res.exec_time_ns} ns")
    out = np.empty((N_NODES, OUT_FEATS), dtype=np.float32)
    for c in range(N_CORES):
        lo, hi = c * SHARD, (c + 1) * SHARD
        out[lo:hi] = res.results[c]["out"][dst_pos[lo:hi]]
    return out


# revision 11
# speedup vs baseline: 1.9842x; 1.9641x over previous
"""MaxK-GCN conv on 8 Trainium2 NeuronCores.

Pipeline (per core c, SPMD over 8 cores; nodes sharded 8 x 12500):
  host: load-balancing permutations — dst nodes LPT-packed into 98 blocks
        (~2045 in-edges each), src nodes greedy-packed into 8 slab-quarters
        so every (slab, block) bucket holds <=256 edges (2 gather tiles).
  phase 1: h = featT_c.T @ W (PE, bf16), top-16-of-64 threshold mask (DVE
           max8 + match_replace), scale by (max(out_deg,1)*max(in_deg,1))^-0.5,
           split each fp32 row into a [hi|lo] bf16 pair -> local table shard
           [12544, 128] bf16 (hi+lo reconstructs fp32 to ~2^-17).
  AllGather the table in 8 slab chunks so phase-2 gathers start after the
  first chunk (~45us) instead of after the full table.
  phase 2: per (pass, slab) group in slab-diagonal order (work tracks
           AllGather availability): dma_gather src rows (SWDGE), one-hot S
           tiles (scalar-engine broadcast-copy + DVE is_eq, keeping DVE in
           2-byte packed mode), matmul S^T @ G per 128-dst block in PSUM,
           evict per-group partial sums into an SBUF accumulator (decouples
           PSUM from pass completion), fold hi+lo + bias after the last
           slab, DMA out.

Edge bookkeeping (permutations, sort, padding, degree counts) is host-side
index metadata; all floating-point math runs on device.
"""
import sys
import os

sys.path.insert(0, "/opt/trn_rl_repo")

import numpy as np
import ml_dtypes
import concourse.bacc as bacc
import concourse.mybir as mybir
import concourse.tile as tile
from concourse.bass_utils import run_bass_kernel_spmd

P = 128
N_NODES = 100000
IN_FEATS = 256
OUT_FEATS = 64
N_CORES = 8
SHARD = N_NODES // N_CORES          # 12500 real nodes per core
SHARD_PAD = 12544                   # 98 * 128
N_BLOCKS = SHARD_PAD // P           # 98
N_SLABS = 4
QROWS = SHARD_PAD // N_SLABS        # 3136 rows per shard-slab
SLAB_ROWS = N_CORES * QROWS         # 25088 rows per slab-table
PASS_BLOCKS = [4] * 24 + [2]        # blocks per PSUM pass
N_PASSES = len(PASS_BLOCKS)
TABLE_ROWS = N_CORES * SHARD_PAD    # 100352
PAIR = 2 * OUT_FEATS                # 128 bf16 per table row (hi|lo)
CTMAX = 24                          # max tiles per (pass, slab) gather call
DIAG_G = 8                          # diagonal lag: calls per AllGather gap
SW = 4                              # one-hot build batch width
NEG_INF = -3.0e38

PASS_OF_BLOCK = np.repeat(np.arange(N_PASSES), PASS_BLOCKS)
PASS_BASE = np.cumsum([0] + PASS_BLOCKS[:-1])


def _balance_perms(src, dst, in_deg, out_deg):
    """Load-balancing layout: per-core dst->block LPT + src->slab greedy.

    Returns (dst_pos, src_pos): position of each node within its core's
    padded shard, for the output rows (dst_pos) and table rows (src_pos).
    """
    import heapq

    dst_pos = np.zeros(N_NODES, dtype=np.int64)
    for c in range(N_CORES):
        lo = c * SHARD
        deg = in_deg[lo:lo + SHARD]
        order = np.argsort(-deg, kind="stable")
        loads = np.zeros(N_BLOCKS, dtype=np.int64)
        slots = np.zeros(N_BLOCKS, dtype=np.int64)
        heap = [(0, b) for b in range(N_BLOCKS)]
        heapq.heapify(heap)
        pos = np.empty(SHARD, dtype=np.int64)
        for v in order:
            while True:
                _, b = heapq.heappop(heap)
                if slots[b] < P:
                    break
            pos[v] = b * P + slots[b]
            slots[b] += 1
            loads[b] += deg[v]
            if slots[b] < P:
                heapq.heappush(heap, (loads[b], b))
        dst_pos[lo:lo + SHARD] = pos

    # src->slab greedy: balance each consumer bucket (dst core, block)
    # across the slab-tables; cells <= cap keep buckets at cap/128 tiles.
    edge_bucket = ((dst // SHARD) * N_BLOCKS + (dst_pos[dst] // P)).astype(np.int64)
    nbkt = N_CORES * N_BLOCKS
    cell = np.zeros((nbkt, N_SLABS), dtype=np.int32)
    cap = (2048 // N_SLABS // P) * P
    src_slab = np.zeros(N_NODES, dtype=np.int8)
    order_e = np.argsort(src, kind="stable")
    sb = edge_bucket[order_e]
    s_sorted = src[order_e]
    starts = np.searchsorted(s_sorted, np.arange(N_NODES))
    ends = np.searchsorted(s_sorted, np.arange(N_NODES) + 1)
    for c in range(N_CORES):
        lo = c * SHARD
        node_order = np.argsort(-out_deg[lo:lo + SHARD], kind="stable") + lo
        qcap = np.full(N_SLABS, QROWS, dtype=np.int64)
        for v in node_order:
            bkts = sb[starts[v]:ends[v]]
            if len(bkts):
                loads = cell[bkts]
                penalty = (np.maximum(loads + 1 - cap, 0) * 1000 + loads).sum(axis=0)
            else:
                penalty = np.zeros(N_SLABS)
            penalty = penalty + (qcap == 0) * 1e12
            q = int(np.argmin(penalty))
            src_slab[v] = q
            qcap[q] -= 1
            if len(bkts):
                np.add.at(cell, (bkts, q), 1)

    # repair: move one contributor out of each overflowing cell when possible,
    # respecting per-core slab row capacities
    fill = np.zeros((N_CORES, N_SLABS), dtype=np.int64)
    for c in range(N_CORES):
        fill[c] = np.bincount(src_slab[c * SHARD:(c + 1) * SHARD],
                              minlength=N_SLABS)
    eq_slab = src_slab[src]
    for _ in range(3):
        over = np.argwhere(cell > cap)
        if not len(over):
            break
        for bkt, q in over:
            if cell[bkt, q] <= cap:
                continue
            cand = np.unique(src[(edge_bucket == bkt) & (eq_slab == q)])
            moved = False
            for v in cand:
                vc = int(v) // SHARD
                bkts = sb[starts[v]:ends[v]]
                for q2 in range(N_SLABS):
                    if q2 == q or fill[vc, q2] >= QROWS:
                        continue
                    ub, mult = np.unique(bkts, return_counts=True)
                    if np.all(cell[ub, q2] + mult <= cap):
                        np.add.at(cell, (bkts, q), -1)
                        np.add.at(cell, (bkts, q2), 1)
                        src_slab[v] = q2
                        fill[vc, q] -= 1
                        fill[vc, q2] += 1
                        eq_slab = src_slab[src]
                        moved = True
                        break
                if moved:
                    break

    src_pos = np.zeros(N_NODES, dtype=np.int64)
    for c in range(N_CORES):
        lo = c * SHARD
        qs = src_slab[lo:lo + SHARD]
        if np.bincount(qs, minlength=N_SLABS).max() > QROWS:
            qs = np.repeat(np.arange(N_SLABS), QROWS)[:SHARD].astype(np.int8)
        fill = np.zeros(N_SLABS, dtype=np.int64)
        pos = np.empty(SHARD, dtype=np.int64)
        for i in range(SHARD):
            q = int(qs[i])
            pos[i] = q * QROWS + fill[q]
            fill[q] += 1
        src_pos[lo:lo + SHARD] = pos
    return dst_pos, src_pos


def _inspect(src, dst, dst_pos, src_pos):
    """Host inspector: per-core sorted edge data + shared static tile grid."""
    core = dst // SHARD
    e_blk = dst_pos[dst] >> 7
    e_rel = dst_pos[dst] & (P - 1)
    e_s8 = src // SHARD
    e_slab = src_pos[src] // QROWS
    e_gidx = e_s8 * QROWS + (src_pos[src] - e_slab * QROWS)
    gidx_of, dstrel_of = [], []
    counts = np.zeros((N_CORES, N_PASSES, N_SLABS, N_BLOCKS), dtype=np.int64)
    for c in range(N_CORES):
        m = core == c
        blk = e_blk[m]
        slab = e_slab[m]
        gidx = e_gidx[m]
        pss = PASS_OF_BLOCK[blk]
        diagkey = slab * DIAG_G + pss
        order = np.lexsort((gidx, blk, slab, diagkey))
        gidx_of.append(gidx[order])
        dstrel_of.append(e_rel[m][order])
        key = (pss * N_SLABS + slab) * N_BLOCKS + blk
        cnt = np.bincount(key, minlength=N_PASSES * N_SLABS * N_BLOCKS)
        counts[c] = cnt.reshape(N_PASSES, N_SLABS, N_BLOCKS)
    T = ((counts + P - 1) // P).max(axis=0)  # shared tile grid
    return gidx_of, dstrel_of, counts, T


def _make_schedule(T):
    """Diagonal (pass, slab) stream: work order tracks AllGather availability."""
    diag = sorted(((p, s) for p in range(N_PASSES) for s in range(N_SLABS)),
                  key=lambda ps: (ps[1] * DIAG_G + ps[0], ps[1]))
    tile_meta = []   # [p, s, b, start, stop]
    calls = []       # (s, j0, ct) one per (p, s) group
    groups = []      # (p, s) in stream order
    for (p, s) in diag:
        j0 = len(tile_meta)
        for b in range(PASS_BASE[p], PASS_BASE[p] + PASS_BLOCKS[p]):
            n = int(T[p, s, b])
            assert n >= 1, (p, s, b)
            for k in range(n):
                tile_meta.append([p, s, b, k == 0, k == n - 1])
        ct = len(tile_meta) - j0
        assert ct <= CTMAX, ct
        calls.append((s, j0, ct))
        groups.append((p, s))
    return tile_meta, calls, groups


def _per_core_streams(c, tile_meta, counts, gidx_of, dstrel_of):
    """This core's padded gather-idx + dst_rel streams matching the grid."""
    ntiles = len(tile_meta)
    idx_stream = np.zeros(ntiles * P, dtype=np.int16)
    dst_stream = np.full(ntiles * P, -1.0, dtype=np.float32)
    edge_ptr = 0
    j = 0
    while j < ntiles:
        p, s, b = tile_meta[j][:3]
        k = j
        while k < ntiles and tile_meta[k][:3] == [p, s, b]:
            k += 1
        nseg = int(counts[c, p, s, b])
        base = j * P
        idx_stream[base:base + nseg] = gidx_of[c][edge_ptr:edge_ptr + nseg]
        dst_stream[base:base + nseg] = dstrel_of[c][edge_ptr:edge_ptr + nseg]
        edge_ptr += nseg
        j = k
    assert edge_ptr == len(gidx_of[c])
    idx_wrapped = np.tile(idx_stream.reshape(-1, 16).T, (8, 1)).copy()
    dstv = dst_stream.reshape(ntiles, P).T.copy()
    return idx_wrapped, dstv


def _build(tile_meta, calls, groups):
    ntiles = len(tile_meta)
    nc = bacc.Bacc("TRN2", target_bir_lowering=False, num_swdge_queues=4)
    dt = mybir.dt

    featT = nc.declare_dram_parameter("featT", [IN_FEATS, SHARD_PAD], dt.float32, isOutput=False)
    w_in = nc.declare_dram_parameter("w", [IN_FEATS, OUT_FEATS], dt.float32, isOutput=False)
    biasb = nc.declare_dram_parameter("biasb", [P, OUT_FEATS], dt.float32, isOutput=False)
    idegw = nc.declare_dram_parameter("idegw", [P, N_BLOCKS], dt.float32, isOutput=False)
    odegw = nc.declare_dram_parameter("odegw", [P, N_BLOCKS], dt.float32, isOutput=False)
    iota_in = nc.declare_dram_parameter("iota", [P, P], dt.bfloat16, isOutput=False)
    idxs_in = nc.declare_dram_parameter("idxs", [P, ntiles * 8], dt.int16, isOutput=False)
    dstv_in = nc.declare_dram_parameter("dstv", [P, ntiles], dt.bfloat16, isOutput=False)
    out_d = nc.declare_dram_parameter("out", [SHARD_PAD, OUT_FEATS], dt.float32, isOutput=True)

    tableL = nc.dram_tensor("tableL", [SHARD_PAD, PAIR], dt.bfloat16)
    tableQ = [nc.dram_tensor(f"tableQ{q}", [SLAB_ROWS, PAIR], dt.bfloat16,
                             addr_space="Shared") for q in range(N_SLABS)]

    with tile.TileContext(nc) as tc:
        with tc.tile_pool(name="const", bufs=1) as constp, \
             tc.tile_pool(name="gp", bufs=10) as gp, \
             tc.tile_pool(name="dmp", bufs=8) as dmp, \
             tc.tile_pool(name="sp", bufs=8) as sps, \
             tc.tile_pool(name="outp", bufs=4) as outp:

            # ---- constants ----
            w_sb = constp.tile([P, 2, OUT_FEATS], dt.float32)
            for k in range(2):
                nc.sync.dma_start(out=w_sb[:, k, :], in_=w_in[k * P:(k + 1) * P, :])
            bias_sb = constp.tile([P, 1, OUT_FEATS], dt.float32)
            nc.sync.dma_start(out=bias_sb[:, 0, :], in_=biasb[:])
            iota4 = constp.tile([P, SW, P], dt.bfloat16)
            for k in range(SW):
                nc.sync.dma_start(out=iota4[:, k, :], in_=iota_in[:])
            dstv_sb = constp.tile([P, ntiles, 1], dt.bfloat16)
            nc.sync.dma_start(out=dstv_sb[:, :, 0], in_=dstv_in[:])
            idx_sb = constp.tile([P, ntiles * 8], dt.int16)
            nc.sync.dma_start(out=idx_sb[:], in_=idxs_in[:])

            # ---- phase 1: table build (pools scoped to free SBUF/PSUM) ----
            with tc.tile_pool(name="ft", bufs=1) as ftp, \
                 tc.tile_pool(name="ph1", bufs=4) as ph1, \
                 tc.tile_pool(name="ph1ps", bufs=4, space="PSUM") as ph1ps:

                ideg_sb = ph1.tile([P, N_BLOCKS], dt.float32, tag="deg")
                odeg_sb = ph1.tile([P, N_BLOCKS], dt.float32, tag="deg")
                nc.sync.dma_start(out=ideg_sb[:], in_=idegw[:])
                nc.sync.dma_start(out=odeg_sb[:], in_=odegw[:])
                scale_sb = constp.tile([P, N_BLOCKS], dt.float32)
                nc.vector.tensor_scalar_max(ideg_sb[:], ideg_sb[:], 1.0)
                nc.vector.tensor_scalar_max(odeg_sb[:], odeg_sb[:], 1.0)
                nc.vector.tensor_mul(out=scale_sb[:], in0=ideg_sb[:], in1=odeg_sb[:])
                nc.scalar.activation(out=scale_sb[:], in_=scale_sb[:],
                                     func=mybir.ActivationFunctionType.Sqrt)
                nc.vector.reciprocal(out=scale_sb[:], in_=scale_sb[:])

                # featT in chunks (2 k-chunks x 8 column chunks)
                FCH = [13] * 7 + [7]
                FBASE = [0, 13, 26, 39, 52, 65, 78, 91]
                ft_sb = {}
                for fc in range(8):
                    for k in range(2):
                        t_ = ftp.tile([P, FCH[fc] * P], dt.float32, tag=f"ft{k}", bufs=2)
                        nc.sync.dma_start(
                            out=t_[:],
                            in_=featT[k * P:(k + 1) * P,
                                      FBASE[fc] * P:(FBASE[fc] + FCH[fc]) * P])
                        ft_sb[(fc, k)] = t_

                for t in range(N_BLOCKS):
                    fc = min(t // 13, 7)
                    tc_rel = t - FBASE[fc]
                    hp = ph1ps.tile([P, OUT_FEATS], dt.float32, tag="hps")
                    for k in range(2):
                        nc.tensor.matmul(
                            out=hp[:],
                            lhsT=ft_sb[(fc, k)][:, tc_rel * P:(tc_rel + 1) * P],
                            rhs=w_sb[:, k, :],
                            start=(k == 0), stop=(k == 1),
                        )
                    h = ph1.tile([P, OUT_FEATS], dt.float32, tag="h")
                    nc.vector.tensor_copy(out=h[:], in_=hp[:])
                    m1 = ph1.tile([P, 8], dt.float32, tag="m1")
                    nc.vector.max(m1[:], h[:])
                    hneg = ph1.tile([P, OUT_FEATS], dt.float32, tag="hneg")
                    nc.vector.match_replace(out=hneg[:], in_to_replace=m1[:],
                                            in_values=h[:], imm_value=NEG_INF)
                    m2 = ph1.tile([P, 8], dt.float32, tag="m2")
                    nc.vector.max(m2[:], hneg[:])
                    # hm = (h >= thr) * h  in one fused op
                    hm = ph1.tile([P, OUT_FEATS], dt.float32, tag="mask")
                    nc.vector.scalar_tensor_tensor(
                        out=hm[:], in0=h[:], scalar=m2[:, 7:8], in1=h[:],
                        op0=mybir.AluOpType.is_ge, op1=mybir.AluOpType.mult)
                    ttile = ph1.tile([P, PAIR], dt.bfloat16, tag="ttile")
                    hi32 = ph1.tile([P, OUT_FEATS], dt.float32, tag="hi32")
                    # hi = bf16(hm * scale) via ACT's fused input scale
                    nc.scalar.activation(out=ttile[:, 0:OUT_FEATS], in_=hm[:],
                                         func=mybir.ActivationFunctionType.Copy,
                                         scale=scale_sb[:, t:t + 1])
                    nc.scalar.activation(out=hi32[:], in_=ttile[:, 0:OUT_FEATS],
                                         func=mybir.ActivationFunctionType.Copy)
                    # lo = bf16(hm * scale - hi32) in one fused op
                    nc.vector.scalar_tensor_tensor(
                        out=ttile[:, OUT_FEATS:PAIR], in0=hm[:],
                        scalar=scale_sb[:, t:t + 1], in1=hi32[:],
                        op0=mybir.AluOpType.mult,
                        op1=mybir.AluOpType.subtract)
                    nc.sync.dma_start(out=tableL[t * P:(t + 1) * P, :], in_=ttile[:])

            # ---- allgather table, one collective per slab (8 chunks) so
            # phase-2 gathers start after the first chunk ----
            for q in range(N_SLABS):
                nc.gpsimd.collective_compute(
                    "AllGather",
                    mybir.AluOpType.bypass,
                    replica_groups=[list(range(N_CORES))],
                    ins=[tableL[q * QROWS:(q + 1) * QROWS, :]],
                    outs=[tableQ[q][:]],
                )

            # ---- phase 2: edge aggregation, diagonal (pass, slab) order ----
            phase2_stack = __import__("contextlib").ExitStack()
            accp = phase2_stack.enter_context(
                tc.tile_pool(name="accp", bufs=2, space="PSUM"))
            oap = phase2_stack.enter_context(
                tc.tile_pool(name="oap", bufs=N_PASSES))
            slab_seen = {}
            pass_acc = {}
            for gi, ((p, s), (s_, j0, ct)) in enumerate(zip(groups, calls)):
                nblk = PASS_BLOCKS[p]
                g = gp.tile([P, CTMAX, PAIR], dt.bfloat16, tag="g")
                nc.gpsimd.dma_gather(
                    out_ap=g[:, :ct, :],
                    in_ap=tableQ[s][:],
                    idxs_ap=idx_sb[:, j0 * 8:(j0 + ct) * 8],
                    num_idxs=ct * P,
                    num_idxs_reg=ct * P,
                    elem_size=PAIR,
                    single_packet=False,
                    queue_num=s % 4,
                )
                # one-hot S tiles: scalar engine materializes dstv (broadcast
                # copy), DVE compares against a packed iota (2-byte mode)
                s_tiles = []
                for t0 in range(0, ct, SW):
                    jn = min(SW, ct - t0)
                    dm = dmp.tile([P, SW, P], dt.bfloat16, tag="dm")
                    nc.scalar.copy(
                        out=dm[:, :jn, :],
                        in_=dstv_sb[:, j0 + t0:j0 + t0 + jn, :].to_broadcast(
                            [P, jn, P]))
                    s4 = sps.tile([P, SW, P], dt.bfloat16, tag="s")
                    nc.vector.tensor_tensor(
                        out=s4[:, :jn, :], in0=dm[:, :jn, :],
                        in1=iota4[:, :jn, :],
                        op=mybir.AluOpType.is_equal)
                    for t in range(jn):
                        s_tiles.append((s4, t))
                acc = accp.tile([P, 4, 512], dt.float32, tag="acc")
                for t in range(ct):
                    p_, s2, b, st, sp_ = tile_meta[j0 + t]
                    b_rel = b - PASS_BASE[p]
                    s4_t, s4_i = s_tiles[t]
                    nc.tensor.matmul(
                        out=acc[:, b_rel, 0:PAIR],
                        lhsT=s4_t[:, s4_i, :],
                        rhs=g[:, t, :],
                        start=bool(st), stop=bool(sp_),
                        skip_group_check=True,
                    )
                # evict group partials into this pass's SBUF accumulator
                B = PASS_BASE[p]
                if p not in pass_acc:
                    pa = oap.tile([P, 4, PAIR], dt.float32, tag="pa")
                    nc.vector.tensor_copy(out=pa[:, :nblk, :],
                                          in_=acc[:, :nblk, 0:PAIR])
                    pass_acc[p] = pa
                else:
                    pa = pass_acc[p]
                    nc.vector.tensor_add(
                        out=pa[:, :nblk, :],
                        in0=pa[:, :nblk, :],
                        in1=acc[:, :nblk, 0:PAIR])
                seen = slab_seen.get(p, 0) + 1
                slab_seen[p] = seen
                if seen == N_SLABS:
                    # final fold: hi + lo + bias, then DMA out
                    hs = outp.tile([P, 4, OUT_FEATS], dt.float32, tag="hs")
                    nc.vector.tensor_add(
                        out=hs[:, :nblk, :],
                        in0=pa[:, :nblk, 0:OUT_FEATS],
                        in1=pa[:, :nblk, OUT_FEATS:PAIR])
                    nc.vector.tensor_add(
                        out=hs[:, :nblk, :],
                        in0=hs[:, :nblk, :],
                        in1=bias_sb[:].to_broadcast([P, nblk, OUT_FEATS]))
                    for b_rel in range(nblk):
                        b = B + b_rel
                        nc.sync.dma_start(out=out_d[b * P:(b + 1) * P, :],
                                          in_=hs[:, b_rel, :])
            phase2_stack.close()

    nc.finalize()
    return nc


def kernel(feat, weight, bias, src, dst):
    feat = np.asarray(feat, dtype=np.float32)
    weight = np.asarray(weight, dtype=np.float32)
    bias = np.asarray(bias, dtype=np.float32)
    src = np.asarray(src)
    dst = np.asarray(dst)

    src64 = src.astype(np.int64)
    dst64 = dst.astype(np.int64)
    in_deg = np.bincount(dst64, minlength=N_NODES).astype(np.float32)
    out_deg = np.bincount(src64, minlength=N_NODES).astype(np.float32)
    dst_pos, src_pos = _balance_perms(src64, dst64,
                                      np.bincount(dst64, minlength=N_NODES),
                                      np.bincount(src64, minlength=N_NODES))
    gidx_of, dstrel_of, counts, T = _inspect(src64, dst64, dst_pos, src_pos)
    tile_meta, calls, groups = _make_schedule(T)

    ft = feat.T  # [256, 100000]
    iota = np.tile(np.arange(P, dtype=np.float32), (P, 1)).astype(ml_dtypes.bfloat16)

    in_maps = []
    for c in range(N_CORES):
        lo, hi = c * SHARD, (c + 1) * SHARD
        featT_c = np.zeros((IN_FEATS, SHARD_PAD), dtype=np.float32)
        featT_c[:, src_pos[lo:hi]] = ft[:, lo:hi]
        ideg_c = np.ones(SHARD_PAD, dtype=np.float32)
        odeg_c = np.ones(SHARD_PAD, dtype=np.float32)
        ideg_c[src_pos[lo:hi]] = in_deg[lo:hi]
        odeg_c[src_pos[lo:hi]] = out_deg[lo:hi]
        idx_wrapped, dstv = _per_core_streams(c, tile_meta, counts,
                                              gidx_of, dstrel_of)
        in_maps.append({
            "featT": featT_c,
            "w": weight,
            "biasb": np.tile(bias[None, :], (P, 1)).astype(np.float32),
            "idegw": ideg_c.reshape(N_BLOCKS, P).T.copy(),
            "odegw": odeg_c.reshape(N_BLOCKS, P).T.copy(),
            "iota": iota,
            "idxs": idx_wrapped,
            "dstv": dstv.astype(ml_dtypes.bfloat16),
        })

    nc = _build(tile_meta, calls, groups)
    res = run_bass_kernel_spmd(nc, in_maps, list(range(N_CORES)),
                               trace=bool(os.environ.get("KERNEL_TRACE")))
    if os.environ.get("KERNEL_TRACE"):
        print(f"HW exec time: {res.exec_time_ns} ns")
    out = np.empty((N_NODES, OUT_FEATS), dtype=np.float32)
    for c in range(N_CORES):
        lo, hi = c * SHARD, (c + 1) * SHARD
        out[lo:hi] = res.results[c]["out"][dst_pos[lo:hi]]
    return out
